# revision 1
# baseline (speedup 1.0000x reference)
"""Trainium2 Bass kernel for the counting-criterion loss.

Computes, for output/density_map of shape [32, 1, 512, 512] and bboxes [32, 3, 4]:
  dmap_loss  = sum((output - density_map)^2) / num_objects
  count_loss = mean_b((sum(output_b) - sum(density_map_b))^2)
  min_count  = sum_boxes(relu(1 - box_sum))   with box sums over [y1:y2, x1:x2)

Strategy: data-parallel over the batch — core i handles images [4i, 4i+4).
On each core, per image:
  - one DVE scalar_tensor_tensor gives diff = o - d plus per-partition sum(diff)
  - one ACT Square activation with accum_out gives per-partition sum(diff^2)
  - box sums via PE: for each x-chunk, O_chunk^T (stationary [128,128]) @
    rowmask (moving [128,3]) accumulated over the 4 y-chunks -> psum[x, (c,j)];
    multiply by the column mask on DVE, then a ones-vector matmul reduces over
    the x partitions.
Final tiny reductions (cross-partition sums, relu, squares, weights) run on
the host from each core's [128,4]+[128,4]+[1,48] partial outputs.
"""

import numpy as np
from contextlib import ExitStack

import concourse.bass as bass
import concourse.mybir as mybir
import concourse.tile as tile
from concourse import bacc
from concourse.bass_utils import run_bass_kernel_spmd

N_CORES = 8
B, H, W = 32, 512, 512
NIMG = B // N_CORES  # images per core
P = 128              # SBUF partitions
NCH = H // P         # row chunks per image (and col chunks: W//P)
NB = 3               # boxes per image
F32 = mybir.dt.float32

_PROG = None


def _build_program():
    nc = bacc.Bacc(
        "TRN2",
        target_bir_lowering=False,
        debug=False,
        num_devices=N_CORES,
    )
    o_d = nc.dram_tensor("o", [NIMG, H, W], F32, kind="ExternalInput").ap()
    d_d = nc.dram_tensor("d", [NIMG, H, W], F32, kind="ExternalInput").ap()
    # packed masks per image: cols 0:NCH*NB row mask [y%128, (y//128, j)],
    # cols NCH*NB:2*NCH*NB col mask [x%128, (x//128, j)]
    msk_d = nc.dram_tensor(
        "msk", [NIMG, P, 2 * NCH * NB], F32, kind="ExternalInput"
    ).ap()
    # columns: img0..img2 as 2 halves each, then img3 as 3 quarters + 2
    # eighths; first NCOL are sum(diff) partials, next NCOL are sum(diff^2)
    # partials, then 48 box partials (row 0 only: img-major (img, cx, j))
    NCOL = 2 * (NIMG - 1) + NCH + 1
    NBOXCOL = NIMG * NCH * NB
    acc_d = nc.dram_tensor(
        "acc", [P, 2 * NCOL + NBOXCOL], F32, kind="ExternalOutput"
    ).ap()

    # DRAM views: image rows split as y = c*128 + p  ->  [img, p, c, x]
    o_r = o_d.rearrange("n (c p) x -> n p c x", p=P)
    d_r = d_d.rearrange("n (c p) x -> n p c x", p=P)

    with tile.TileContext(nc) as tc, ExitStack() as ctx:
        io_pool = ctx.enter_context(tc.tile_pool(name="io", bufs=2))
        qio_pool = ctx.enter_context(tc.tile_pool(name="qio", bufs=1))
        mask_pool = ctx.enter_context(tc.tile_pool(name="mask", bufs=2))
        work_pool = ctx.enter_context(tc.tile_pool(name="work", bufs=2))
        psum_pool = ctx.enter_context(tc.tile_pool(name="psum", bufs=2, space="PSUM"))
        acc_pool = ctx.enter_context(tc.tile_pool(name="acc", bufs=1))

        acc = acc_pool.tile([P, 2 * NCOL + NBOXCOL], F32)
        nc.vector.memset(acc[:], 0.0)
        ones_t = acc_pool.tile([P, 1], F32)
        nc.vector.memset(ones_t[:], 1.0)

        def box_work(img, o_chunks, msk_t):
            """o_chunks: list of (tile, free-index) giving [128, 512] y-chunk APs."""
            ps = psum_pool.tile([P, NCH * NB], F32, tag="ps")
            for cx in range(NCH):
                for cy in range(NCH):
                    t, idx = o_chunks[cy]
                    nc.tensor.matmul(
                        ps[:, cx * NB : (cx + 1) * NB],
                        lhsT=t[:, idx, cx * P : (cx + 1) * P],
                        rhs=msk_t[:, cy * NB : (cy + 1) * NB],
                        start=(cy == 0),
                        stop=(cy == NCH - 1),
                    )
            masked_t = work_pool.tile([P, NCH * NB], F32, tag="masked")
            nc.vector.tensor_tensor(
                out=masked_t[:],
                in0=ps[:],
                in1=msk_t[:, NCH * NB : 2 * NCH * NB],
                op=mybir.AluOpType.mult,
            )
            ps2 = psum_pool.tile([1, NCH * NB], F32, tag="ps2")
            nc.tensor.matmul(
                ps2[:], lhsT=ones_t[:], rhs=masked_t[:], start=True, stop=True
            )
            col0 = 2 * NCOL + img * NCH * NB
            nc.vector.tensor_copy(acc[0:1, col0 : col0 + NCH * NB], ps2[:])

        def diff_work(o_ap, d_ap, col, square_on_dve=False, tag=""):
            """stt diff + square over one chunk, accumulating into column col.

            The square runs on ACT by default (hides under DMA); for the tail
            chunks it runs on DVE so the critical chain stays on one engine.
            """
            diff_t = work_pool.tile(
                list(o_ap.shape), F32, tag="diff" + tag, bufs=5 if tag else None
            )
            nc.vector.scalar_tensor_tensor(
                out=diff_t[:],
                in0=o_ap,
                scalar=0.0,
                in1=d_ap,
                op0=mybir.AluOpType.bypass,
                op1=mybir.AluOpType.subtract,
                accum_out=acc[:, col : col + 1],
            )
            sq_t = work_pool.tile(
                list(o_ap.shape), F32, tag="sq" + tag, bufs=5 if tag else None
            )
            if square_on_dve:
                nc.vector.scalar_tensor_tensor(
                    out=sq_t[:],
                    in0=diff_t[:],
                    scalar=0.0,
                    in1=diff_t[:],
                    op0=mybir.AluOpType.bypass,
                    op1=mybir.AluOpType.mult,
                    accum_out=acc[:, NCOL + col : NCOL + col + 1],
                )
            else:
                nc.scalar.activation(
                    sq_t[:],
                    diff_t[:],
                    mybir.ActivationFunctionType.Square,
                    accum_out=acc[:, NCOL + col : NCOL + col + 1],
                )

        msk_all = mask_pool.tile([P, NIMG, 2 * NCH * NB], F32)

        # images 0..NIMG-2: half-image pipeline (keeps DVE/ACT streaming
        # steadily behind the DMA instead of big 2.2us blocks)
        HC = NCH // 2
        for img in range(NIMG - 1):
            halves = []
            for h in range(2):
                o_t = io_pool.tile([P, HC, W], F32, tag=f"o{h}")
                nc.sync.dma_start(o_t[:], o_r[img, :, h * HC : (h + 1) * HC])
                d_t = io_pool.tile([P, HC, W], F32, tag=f"d{h}")
                nc.sync.dma_start(d_t[:], d_r[img, :, h * HC : (h + 1) * HC])
                if img == 0 and h == 0:
                    # all masks in one small DMA, tucked behind the first pair
                    nc.sync.dma_start(
                        msk_all[:], msk_d.rearrange("n p m -> p n m")
                    )
                diff_work(o_t[:], d_t[:], 2 * img + h)
                halves.append(o_t)
            box_work(
                img,
                [(halves[c // HC], c % HC) for c in range(NCH)],
                msk_all[:, img],
            )

        # last image: quarter-chunks with interleaved o/d DMAs (last quarter as
        # two eighths) so the post-DMA tail is only an eighth-image chain
        img = NIMG - 1
        oq_tiles, chunks = [], []
        for c in range(NCH):
            if c < NCH - 1:
                oq = qio_pool.tile([P, 1, W], F32, tag=f"oq{c}")
                nc.sync.dma_start(oq[:], o_r[img, :, c : c + 1])
                dq = qio_pool.tile([P, 1, W], F32, tag=f"dq{c}")
                nc.sync.dma_start(dq[:], d_r[img, :, c : c + 1])
                oq_tiles.append((oq, 0))
                chunks.append((oq[:], dq[:]))
            else:
                # final quarter as two eighth-image pieces
                oq = qio_pool.tile([P, 1, W], F32, tag=f"oq{c}")
                dq = qio_pool.tile([P, 1, W], F32, tag=f"dq{c}")
                for h in range(2):
                    hs = slice(h * (W // 2), (h + 1) * (W // 2))
                    nc.sync.dma_start(oq[:, 0, hs], o_r[img, :, c, hs])
                    nc.sync.dma_start(dq[:, 0, hs], d_r[img, :, c, hs])
                    chunks.append((oq[:, 0, hs], dq[:, 0, hs]))
                oq_tiles.append((oq, 0))
        for i, (o_ap, d_ap) in enumerate(chunks):
            # the very last chunk squares on DVE: keeps the critical chain on
            # one engine with no cross-engine semaphore hop
            diff_work(
                o_ap,
                d_ap,
                2 * (NIMG - 1) + i,
                square_on_dve=(i == len(chunks) - 1),
                tag="q",
            )
        box_work(img, oq_tiles, msk_all[:, img])

        nc.sync.dma_start(acc_d, acc[:])

    nc.compile()
    return nc


def _get_program():
    global _PROG
    if _PROG is None:
        _PROG = _build_program()
    return _PROG


def _prep_inputs(output, density_map, bboxes):
    o = np.ascontiguousarray(np.asarray(output, dtype=np.float32).reshape(B, H, W))
    dm = np.ascontiguousarray(
        np.asarray(density_map, dtype=np.float32).reshape(B, H, W)
    )
    bb = np.clip(np.asarray(bboxes).astype(np.int64), 0, W).astype(np.int32)
    x1, y1, x2, y2 = bb[..., 0], bb[..., 1], bb[..., 2], bb[..., 3]
    x2 = np.maximum(x2, x1)
    y2 = np.maximum(y2, y1)

    ar = np.arange(H, dtype=np.int32)
    # rm[b, y, j] = 1 if y1 <= y < y2, laid out as [b, y%128, (y//128, j)]
    rm = (
        (ar[None, :, None] >= y1[:, None, :]) & (ar[None, :, None] < y2[:, None, :])
    ).astype(np.float32)
    rm = rm.reshape(B, NCH, P, NB).transpose(0, 2, 1, 3).reshape(B, P, NCH * NB)
    # cm[b, j, x] = 1 if x1 <= x < x2, laid out as [b, x%128, (x//128, j)]
    cm = (
        (ar[None, None, :] >= x1[:, :, None]) & (ar[None, None, :] < x2[:, :, None])
    ).astype(np.float32)
    cm = cm.reshape(B, NB, NCH, P).transpose(0, 3, 2, 1).reshape(B, P, NCH * NB)
    msk = np.ascontiguousarray(np.concatenate([rm, cm], axis=2))  # [B, P, 24]
    return o, dm, msk


def kernel(output, density_map, bboxes, num_objects):
    o, dm, msk = _prep_inputs(output, density_map, bboxes)

    nc = _get_program()
    in_maps = [
        {
            "o": o[i * NIMG : (i + 1) * NIMG],
            "d": dm[i * NIMG : (i + 1) * NIMG],
            "msk": msk[i * NIMG : (i + 1) * NIMG],
        }
        for i in range(N_CORES)
    ]
    res = run_bass_kernel_spmd(nc, in_maps, core_ids=list(range(N_CORES)))

    NCOL = 2 * (NIMG - 1) + NCH + 1

    def _per_img(cols):
        # columns: img0..img2 as 2 halves each, img3 as its remaining chunks
        firsts = [cols[2 * i] + cols[2 * i + 1] for i in range(NIMG - 1)]
        return np.array(firsts + [cols[2 * (NIMG - 1) :].sum()])

    per_img_d = np.concatenate(
        [
            _per_img(r["acc"][:, :NCOL].sum(axis=0, dtype=np.float64))
            for r in res.results
        ]
    )  # [B] sum(o - d) per image
    sq_total = float(
        sum(r["acc"][:, NCOL : 2 * NCOL].sum(dtype=np.float64) for r in res.results)
    )  # sum((o - d)^2)
    # acc[0, 2*NCOL + (img, cx, j)] -> sum over cx -> [NIMG, NB], image-major
    box_sums = np.concatenate(
        [
            r["acc"][0, 2 * NCOL :]
            .reshape(NIMG, NCH, NB)
            .sum(axis=1, dtype=np.float64)
            .reshape(-1)
            for r in res.results
        ]
    )  # [B*NB]

    dmap_loss = sq_total / float(num_objects)
    count_loss = float(np.mean(per_img_d**2))
    min_count = float(np.maximum(0.0, 1.0 - box_sums).sum())
    return np.array([dmap_loss, count_loss, min_count], dtype=np.float32)



# revision 4
# speedup vs baseline: 1.4027x; 1.4027x over previous
"""Trainium2 Bass kernel for the counting-criterion loss.

Computes, for output/density_map of shape [32, 1, 512, 512] and bboxes [32, 3, 4]:
  dmap_loss  = sum((output - density_map)^2) / num_objects
  count_loss = mean_b((sum(output_b) - sum(density_map_b))^2)
  min_count  = sum_boxes(relu(1 - box_sum))   with box sums over [y1:y2, x1:x2)

Strategy: data-parallel over the batch -- core i handles images [4i, 4i+4).
Inputs are cast to bf16 on the host (rel-err impact ~3.5e-3, well inside the
2e-2 gate), halving HBM traffic vs f32; the DMA stream is the roofline.
On each core, per image:
  - DVE scalar_tensor_tensor: diff = o - d (bf16) + per-partition sum(diff)
  - ACT Square activation with accum_out: per-partition sum(diff^2)
  - box row-sums on PE: psum[x, (cx,j)] += O_block^T @ rowmask, accumulated
    over the 4 y-chunks; psum copied to the acc tile by DVE.  The column-mask
    multiply and all final tiny reductions happen on the host.
The last transferred piece is a small d-sliver whose diff+square run on DVE
only, so the critical tail is one engine chain plus the final tiny acc DMA.
"""

import numpy as np
import ml_dtypes
from contextlib import ExitStack

import concourse.bass as bass
import concourse.mybir as mybir
import concourse.tile as tile
from concourse import bacc
from concourse.bass_utils import run_bass_kernel_spmd

N_CORES = 8
B, H, W = 32, 512, 512
NIMG = B // N_CORES  # images per core
P = 128              # SBUF partitions
NCH = H // P         # row chunks per image (and col chunks: W//P)
NB = 3               # boxes per image
SLIV = 128           # tail sliver columns (last chunk of last image's d)
F32 = mybir.dt.float32
BF16 = mybir.dt.bfloat16

# stt/sq chunk list: (img, chunk-slice tag); 6 chunks cover everything except
# the tail sliver, which accumulates into the separate acct tile.
NSTT = 6  # img0 h0, img0 h1, img1, img2, img3 c0..2, img3 c3-left
ACC_SQ = NSTT
ACC_BOX = 2 * NSTT
NACC = 2 * NSTT + NIMG * NCH * NB  # 60

_PROG = None


def _build_program():
    nc = bacc.Bacc(
        "TRN2",
        target_bir_lowering=False,
        debug=False,
        num_devices=N_CORES,
    )
    o_d = nc.dram_tensor("o", [NIMG, H, W], BF16, kind="ExternalInput").ap()
    d_d = nc.dram_tensor("d", [NIMG, H, W], BF16, kind="ExternalInput").ap()
    # row masks, host-packed as [p, img, (cy, j)]
    rm_d = nc.dram_tensor("rm", [P, NIMG * NCH * NB], BF16, kind="ExternalInput").ap()
    acc_d = nc.dram_tensor("acc", [P, NACC], F32, kind="ExternalOutput").ap()
    acct_d = nc.dram_tensor("acct", [P, 2], F32, kind="ExternalOutput").ap()

    # DRAM views: image rows split as y = c*128 + p  ->  [img, p, c, x]
    o_r = o_d.rearrange("n (c p) x -> n p c x", p=P)
    d_r = d_d.rearrange("n (c p) x -> n p c x", p=P)

    with tile.TileContext(nc) as tc, ExitStack() as ctx:
        io_pool = ctx.enter_context(tc.tile_pool(name="io", bufs=1))
        work_pool = ctx.enter_context(tc.tile_pool(name="work", bufs=1))
        psum_pool = ctx.enter_context(tc.tile_pool(name="psum", bufs=1, space="PSUM"))
        acc_pool = ctx.enter_context(tc.tile_pool(name="acc", bufs=1))

        acc = acc_pool.tile([P, NACC], F32)
        acct = acc_pool.tile([P, 2], F32)
        warm = acc_pool.tile([P, 1], F32)
        nc.vector.memset(acc[:], 0.0)
        nc.vector.memset(acct[:], 0.0)
        nc.vector.memset(warm[:], 0.0)
        # force the Square act-table load early so it hides under the first DMA
        nc.scalar.activation(warm[:], warm[:], mybir.ActivationFunctionType.Square)

        o_tiles = [io_pool.tile([P, NCH, W], BF16, tag=f"o{i}", name=f"o{i}") for i in range(NIMG)]
        d_tiles = [io_pool.tile([P, NCH, W], BF16, tag=f"d{i}", name=f"d{i}") for i in range(NIMG)]
        rm_t = io_pool.tile([P, NIMG, NCH * NB], BF16, tag="rm")

        def stt(o_ap, d_ap, shape, col, tag):
            """diff = o - d with per-partition accumulated sum -> acc col."""
            diff_t = work_pool.tile([P] + shape, BF16, tag="diff" + tag, name="diff" + tag)
            nc.vector.scalar_tensor_tensor(
                out=diff_t[:],
                in0=o_ap,
                scalar=0.0,
                in1=d_ap,
                op0=mybir.AluOpType.bypass,
                op1=mybir.AluOpType.subtract,
                accum_out=acc[:, col : col + 1],
            )
            return diff_t

        def square(diff_t, col, tag):
            sq_t = work_pool.tile(list(diff_t.shape), BF16, tag="sq" + tag, name="sq" + tag)
            nc.scalar.activation(
                sq_t[:],
                diff_t[:],
                mybir.ActivationFunctionType.Square,
                accum_out=acc[:, ACC_SQ + col : ACC_SQ + col + 1],
            )

        def boxes(img):
            ps = psum_pool.tile([P, NCH * NB], F32, tag=f"ps{img}", name=f"ps{img}")
            for cx in range(NCH):
                for cy in range(NCH):
                    nc.tensor.matmul(
                        ps[:, cx * NB : (cx + 1) * NB],
                        lhsT=o_tiles[img][:, cy, cx * P : (cx + 1) * P],
                        rhs=rm_t[:, img, cy * NB : (cy + 1) * NB],
                        start=(cy == 0),
                        stop=(cy == NCH - 1),
                    )
            col0 = ACC_BOX + img * NCH * NB
            nc.vector.tensor_copy(acc[:, col0 : col0 + NCH * NB], ps[:])

        # --- image 0: half-image pieces so DVE starts early -----------------
        HC = NCH // 2
        for h in range(2):
            nc.sync.dma_start(
                o_tiles[0][:, h * HC : (h + 1) * HC], o_r[0, :, h * HC : (h + 1) * HC]
            )
            nc.sync.dma_start(
                d_tiles[0][:, h * HC : (h + 1) * HC], d_r[0, :, h * HC : (h + 1) * HC]
            )
            if h == 0:
                nc.sync.dma_start(rm_t[:], rm_d)
            dh = stt(
                o_tiles[0][:, h * HC : (h + 1) * HC],
                d_tiles[0][:, h * HC : (h + 1) * HC],
                [HC, W],
                h,
                tag=f"0h{h}",
            )
            square(dh, h, tag=f"0h{h}")
        boxes(0)

        # --- images 1..2: full-image pieces ---------------------------------
        for img in (1, 2):
            nc.sync.dma_start(o_tiles[img][:], o_r[img])
            nc.sync.dma_start(d_tiles[img][:], d_r[img])
            di = stt(
                o_tiles[img][:],
                d_tiles[img][:],
                [NCH, W],
                1 + img,
                tag=f"{img}",
            )
            square(di, 1 + img, tag=f"{img}")
            boxes(img)

        # --- image 3: o full, d split main + sliver (sliver transfers last) -
        img = NIMG - 1
        nc.sync.dma_start(o_tiles[img][:], o_r[img])
        boxes(img)
        nc.sync.dma_start(d_tiles[img][:, 0 : NCH - 1], d_r[img, :, 0 : NCH - 1])
        da = stt(
            o_tiles[img][:, 0 : NCH - 1],
            d_tiles[img][:, 0 : NCH - 1],
            [NCH - 1, W],
            4,
            tag="3a",
        )
        square(da, 4, tag="3a")
        nc.sync.dma_start(
            d_tiles[img][:, NCH - 1, 0 : W - SLIV],
            d_r[img, :, NCH - 1, 0 : W - SLIV],
        )
        db = stt(
            o_tiles[img][:, NCH - 1, 0 : W - SLIV],
            d_tiles[img][:, NCH - 1, 0 : W - SLIV],
            [W - SLIV],
            5,
            tag="3b",
        )
        square(db, 5, tag="3b")

        # sliver transfers last; its diff+square stay on DVE
        nc.sync.dma_start(
            d_tiles[img][:, NCH - 1, W - SLIV : W],
            d_r[img, :, NCH - 1, W - SLIV : W],
        )
        # main acc out via the Activation HWDGE queue so the SP queue stays
        # free for the sliver/acct path
        nc.scalar.dma_start(acc_d, acc[:])

        ds = work_pool.tile([P, SLIV], BF16, tag="diff3s")
        nc.vector.scalar_tensor_tensor(
            out=ds[:],
            in0=o_tiles[img][:, NCH - 1, W - SLIV : W],
            scalar=0.0,
            in1=d_tiles[img][:, NCH - 1, W - SLIV : W],
            op0=mybir.AluOpType.bypass,
            op1=mybir.AluOpType.subtract,
            accum_out=acct[:, 0:1],
        )
        sqs = work_pool.tile([P, SLIV], BF16, tag="sq3s")
        nc.vector.scalar_tensor_tensor(
            out=sqs[:],
            in0=ds[:],
            scalar=0.0,
            in1=ds[:],
            op0=mybir.AluOpType.bypass,
            op1=mybir.AluOpType.mult,
            accum_out=acct[:, 1:2],
        )
        nc.sync.dma_start(acct_d, acct[:])

    nc.compile()
    return nc


def _get_program():
    global _PROG
    if _PROG is None:
        _PROG = _build_program()
    return _PROG


def _prep_inputs(output, density_map, bboxes):
    o = np.asarray(output, dtype=np.float32).reshape(B, H, W).astype(ml_dtypes.bfloat16)
    dm = (
        np.asarray(density_map, dtype=np.float32)
        .reshape(B, H, W)
        .astype(ml_dtypes.bfloat16)
    )
    bb = np.clip(np.asarray(bboxes).astype(np.int64), 0, W).astype(np.int32)
    x1, y1, x2, y2 = bb[..., 0], bb[..., 1], bb[..., 2], bb[..., 3]
    x2 = np.maximum(x2, x1)
    y2 = np.maximum(y2, y1)

    ar = np.arange(H, dtype=np.int32)
    # rm[b, y, j] = 1 if y1 <= y < y2, packed as [b, y%128, (y//128, j)]
    rm = (
        (ar[None, :, None] >= y1[:, None, :]) & (ar[None, :, None] < y2[:, None, :])
    ).astype(np.float32)
    rm = (
        rm.reshape(B, NCH, P, NB)
        .transpose(0, 2, 1, 3)
        .astype(ml_dtypes.bfloat16)  # [b, p, cy, j]
    )
    # col mask stays on the host: cm[b, x, j]
    cm = (
        (ar[None, :, None] >= x1[:, None, :]) & (ar[None, :, None] < x2[:, None, :])
    ).astype(np.float64)
    return o, dm, rm, cm


def kernel(output, density_map, bboxes, num_objects):
    o, dm, rm, cm = _prep_inputs(output, density_map, bboxes)

    nc = _get_program()
    in_maps = [
        {
            "o": np.ascontiguousarray(o[i * NIMG : (i + 1) * NIMG]),
            "d": np.ascontiguousarray(dm[i * NIMG : (i + 1) * NIMG]),
            # [p, img, cy, j] flattened to [p, img*12]
            "rm": np.ascontiguousarray(
                rm[i * NIMG : (i + 1) * NIMG].transpose(1, 0, 2, 3).reshape(P, -1)
            ),
        }
        for i in range(N_CORES)
    ]
    res = run_bass_kernel_spmd(nc, in_maps, core_ids=list(range(N_CORES)))

    per_img_d = []   # [B] sum(o - d) per image
    sq_total = 0.0
    box_sums = []    # [B, NB]
    for ci, r in enumerate(res.results):
        a = r["acc"].astype(np.float64)
        at = r["acct"].astype(np.float64)
        stt_cols = a[:, :NSTT].sum(axis=0)
        sq_cols = a[:, ACC_SQ:ACC_BOX].sum(axis=0)
        sliv_d, sliv_sq = at[:, 0].sum(), at[:, 1].sum()
        per_img_d.extend(
            [
                stt_cols[0] + stt_cols[1],
                stt_cols[2],
                stt_cols[3],
                stt_cols[4] + stt_cols[5] + sliv_d,
            ]
        )
        sq_total += sq_cols.sum() + sliv_sq
        # box partials: [128, (img, cx, j)]; full x index = cx * 128 + p
        bp = a[:, ACC_BOX:].reshape(P, NIMG, NCH, NB)
        for k in range(NIMG):
            img = ci * NIMG + k
            bx = bp[:, k].transpose(1, 0, 2).reshape(W, NB)
            box_sums.append((bx * cm[img]).sum(axis=0))

    dmap_loss = sq_total / float(num_objects)
    count_loss = float(np.mean(np.asarray(per_img_d) ** 2))
    min_count = float(np.maximum(0.0, 1.0 - np.asarray(box_sums)).sum())
    return np.array([dmap_loss, count_loss, min_count], dtype=np.float32)


# revision 6
# speedup vs baseline: 1.4370x; 1.0244x over previous
"""Trainium2 Bass kernel for the counting-criterion loss.

Computes, for output/density_map of shape [32, 1, 512, 512] and bboxes [32, 3, 4]:
  dmap_loss  = sum((output - density_map)^2) / num_objects
  count_loss = mean_b((sum(output_b) - sum(density_map_b))^2)
  min_count  = sum_boxes(relu(1 - box_sum))   with box sums over [y1:y2, x1:x2)

Strategy: data-parallel over the batch -- core i handles images [4i, 4i+4).
Inputs are cast to bf16 on the host (rel-err impact ~3.5e-3, well inside the
2e-2 gate), halving HBM traffic vs f32; the DMA stream is the roofline.
On each core, per image:
  - DVE scalar_tensor_tensor: diff = o - d (bf16) + per-partition sum(diff)
  - ACT Square activation with accum_out: per-partition sum(diff^2)
  - box row-sums on PE: psum[x, (cx,j)] += O_block^T @ rowmask, accumulated
    over the 4 y-chunks; psum copied to the acc tile by DVE.  The column-mask
    multiply and all final tiny reductions happen on the host.
The last image's density map arrives as a tapered sequence of shrinking
pieces; the final two 128-col pieces' diff+square run back-to-back on DVE so
the post-stream critical path is short, ending in a tiny second acc DMA.
"""

import numpy as np
import ml_dtypes
from contextlib import ExitStack

import concourse.bass as bass
import concourse.mybir as mybir
import concourse.tile as tile
from concourse import bacc
from concourse.bass_utils import run_bass_kernel_spmd

N_CORES = 8
B, H, W = 32, 512, 512
NIMG = B // N_CORES  # images per core
P = 128              # SBUF partitions
NCH = H // P         # row chunks per image (and col chunks: W//P)
NB = 3               # boxes per image
F32 = mybir.dt.float32
BF16 = mybir.dt.bfloat16

# stt accumulator columns (acc): img0 h0/h1, img1 h0/h1, img2 h0/h1,
# img3 h0 (c0c1), img3 c2, img3 c3[0:256]       -> 9
# sq accumulator columns (acc): img0, img1, img2, img3 c0c1, img3 c2,
# img3 c3[0:256]                                 -> 6
# box psum copies: 12 per image                  -> 48
NSTT = 9
NSQ = 6
ACC_SQ = NSTT
ACC_BOX = NSTT + NSQ
NACC = ACC_BOX + NIMG * NCH * NB  # 63
# acct columns: img3 c3[256:384] stt/sq, img3 c3[384:512] stt/sq
NACCT = 4

_PROG = None


def _build_program():
    nc = bacc.Bacc(
        "TRN2",
        target_bir_lowering=False,
        debug=False,
        num_devices=N_CORES,
    )
    o_d = nc.dram_tensor("o", [NIMG, H, W], BF16, kind="ExternalInput").ap()
    d_d = nc.dram_tensor("d", [NIMG, H, W], BF16, kind="ExternalInput").ap()
    # row masks, host-packed as [p, (img, cy, j)]
    rm_d = nc.dram_tensor("rm", [P, NIMG * NCH * NB], BF16, kind="ExternalInput").ap()
    acc_d = nc.dram_tensor("acc", [P, NACC], F32, kind="ExternalOutput").ap()
    acct_d = nc.dram_tensor("acct", [P, NACCT], F32, kind="ExternalOutput").ap()

    # DRAM views: image rows split as y = c*128 + p  ->  [img, p, c, x]
    o_r = o_d.rearrange("n (c p) x -> n p c x", p=P)
    d_r = d_d.rearrange("n (c p) x -> n p c x", p=P)

    with tile.TileContext(nc) as tc, ExitStack() as ctx:
        io_pool = ctx.enter_context(tc.tile_pool(name="io", bufs=1))
        work_pool = ctx.enter_context(tc.tile_pool(name="work", bufs=1))
        psum_pool = ctx.enter_context(tc.tile_pool(name="psum", bufs=1, space="PSUM"))
        acc_pool = ctx.enter_context(tc.tile_pool(name="acc", bufs=1))

        acc = acc_pool.tile([P, NACC], F32)
        acct = acc_pool.tile([P, NACCT], F32)
        warm = acc_pool.tile([P, 1], F32)
        nc.vector.memset(acc[:], 0.0)
        nc.vector.memset(acct[:], 0.0)
        nc.vector.memset(warm[:], 0.0)
        # force the Square act-table load early so it hides under the first DMA
        nc.scalar.activation(warm[:], warm[:], mybir.ActivationFunctionType.Square)

        o_tiles = [
            io_pool.tile([P, NCH, W], BF16, tag=f"o{i}", name=f"o{i}")
            for i in range(NIMG)
        ]
        d_tiles = [
            io_pool.tile([P, NCH, W], BF16, tag=f"d{i}", name=f"d{i}")
            for i in range(NIMG)
        ]
        diff_tiles = [
            work_pool.tile([P, NCH, W], BF16, tag=f"f{i}", name=f"f{i}")
            for i in range(NIMG)
        ]
        sq_tiles = [
            work_pool.tile([P, NCH, W], BF16, tag=f"s{i}", name=f"s{i}")
            for i in range(NIMG)
        ]
        rm_t = io_pool.tile([P, NIMG, NCH * NB], BF16, tag="rm")

        def stt_piece(img, ap_slice, col, on_acct=False):
            """diff = o - d over a piece; per-partition sum -> acc/acct col."""
            tgt = acct if on_acct else acc
            nc.vector.scalar_tensor_tensor(
                out=diff_tiles[img][ap_slice],
                in0=o_tiles[img][ap_slice],
                scalar=0.0,
                in1=d_tiles[img][ap_slice],
                op0=mybir.AluOpType.bypass,
                op1=mybir.AluOpType.subtract,
                accum_out=tgt[:, col : col + 1],
            )

        def sq_act(img, ap_slice, col):
            nc.scalar.activation(
                sq_tiles[img][ap_slice],
                diff_tiles[img][ap_slice],
                mybir.ActivationFunctionType.Square,
                accum_out=acc[:, ACC_SQ + col : ACC_SQ + col + 1],
            )

        def sq_dve(img, ap_slice, col):
            nc.vector.scalar_tensor_tensor(
                out=sq_tiles[img][ap_slice],
                in0=diff_tiles[img][ap_slice],
                scalar=0.0,
                in1=diff_tiles[img][ap_slice],
                op0=mybir.AluOpType.bypass,
                op1=mybir.AluOpType.mult,
                accum_out=acct[:, col : col + 1],
            )

        def boxes(img):
            ps = psum_pool.tile([P, NCH * NB], F32, tag=f"ps{img}", name=f"ps{img}")
            for cx in range(NCH):
                for cy in range(NCH):
                    nc.tensor.matmul(
                        ps[:, cx * NB : (cx + 1) * NB],
                        lhsT=o_tiles[img][:, cy, cx * P : (cx + 1) * P],
                        rhs=rm_t[:, img, cy * NB : (cy + 1) * NB],
                        start=(cy == 0),
                        stop=(cy == NCH - 1),
                    )
            return ps

        def ps_copy(img, ps):
            col0 = ACC_BOX + img * NCH * NB
            nc.vector.tensor_copy(acc[:, col0 : col0 + NCH * NB], ps[:])

        HC = NCH // 2
        lo = np.s_[:, 0:HC]          # chunks 0..1
        hi = np.s_[:, HC:NCH]        # chunks 2..3

        # --- images 0..2: o/d half-image DMAs, per-half stt, per-image sq ---
        for img in range(NIMG - 1):
            for h, sl in enumerate((lo, hi)):
                dsl = np.s_[img, :, h * HC : (h + 1) * HC]
                nc.sync.dma_start(o_tiles[img][sl], o_r[dsl])
                nc.sync.dma_start(d_tiles[img][sl], d_r[dsl])
                if img == 0 and h == 0:
                    nc.sync.dma_start(rm_t[:], rm_d)
                stt_piece(img, sl, 2 * img + h)
            sq_act(img, np.s_[:], img)
            ps_copy(img, boxes(img))

        # --- image 3: o full; d tapered c0c1 / c2 / 256 / 128 / 128 ---------
        img = NIMG - 1
        nc.sync.dma_start(o_tiles[img][:], o_r[img])
        nc.sync.dma_start(d_tiles[img][lo], d_r[img, :, 0:HC])
        stt_piece(img, lo, 6)
        sq_act(img, lo, 3)
        ps_copy(img, boxes(img))
        c2 = np.s_[:, 2, 0:W]
        nc.sync.dma_start(d_tiles[img][c2], d_r[img, :, 2, 0:W])
        stt_piece(img, c2, 7)
        sq_act(img, c2, 4)
        c3a = np.s_[:, 3, 0:256]
        nc.sync.dma_start(d_tiles[img][c3a], d_r[img, :, 3, 0:256])
        stt_piece(img, c3a, 8)
        sq_act(img, c3a, 5)
        # main acc out on the ACT HWDGE queue (waits for sq c3a + copies)
        nc.scalar.dma_start(acc_d, acc[:])

        c3b = np.s_[:, 3, 256:384]
        nc.sync.dma_start(d_tiles[img][c3b], d_r[img, :, 3, 256:384])
        stt_piece(img, c3b, 0, on_acct=True)
        sq_dve(img, c3b, 1)
        c3s = np.s_[:, 3, 384:512]
        nc.sync.dma_start(d_tiles[img][c3s], d_r[img, :, 3, 384:512])
        stt_piece(img, c3s, 2, on_acct=True)
        sq_dve(img, c3s, 3)
        nc.sync.dma_start(acct_d, acct[:])

    nc.compile()
    return nc


def _get_program():
    global _PROG
    if _PROG is None:
        _PROG = _build_program()
    return _PROG


def _prep_inputs(output, density_map, bboxes):
    o = np.asarray(output, dtype=np.float32).reshape(B, H, W).astype(ml_dtypes.bfloat16)
    dm = (
        np.asarray(density_map, dtype=np.float32)
        .reshape(B, H, W)
        .astype(ml_dtypes.bfloat16)
    )
    bb = np.clip(np.asarray(bboxes).astype(np.int64), 0, W).astype(np.int32)
    x1, y1, x2, y2 = bb[..., 0], bb[..., 1], bb[..., 2], bb[..., 3]
    x2 = np.maximum(x2, x1)
    y2 = np.maximum(y2, y1)

    ar = np.arange(H, dtype=np.int32)
    # rm[b, y, j] = 1 if y1 <= y < y2, packed as [b, p, (cy, j)]
    rm = (
        (ar[None, :, None] >= y1[:, None, :]) & (ar[None, :, None] < y2[:, None, :])
    ).astype(np.float32)
    rm = rm.reshape(B, NCH, P, NB).transpose(0, 2, 1, 3).astype(ml_dtypes.bfloat16)
    # col mask stays on the host: cm[b, x, j]
    cm = (
        (ar[None, :, None] >= x1[:, None, :]) & (ar[None, :, None] < x2[:, None, :])
    ).astype(np.float64)
    return o, dm, rm, cm


def kernel(output, density_map, bboxes, num_objects):
    o, dm, rm, cm = _prep_inputs(output, density_map, bboxes)

    nc = _get_program()
    in_maps = [
        {
            "o": np.ascontiguousarray(o[i * NIMG : (i + 1) * NIMG]),
            "d": np.ascontiguousarray(dm[i * NIMG : (i + 1) * NIMG]),
            # [p, img, cy, j] flattened to [p, img*12]
            "rm": np.ascontiguousarray(
                rm[i * NIMG : (i + 1) * NIMG].transpose(1, 0, 2, 3).reshape(P, -1)
            ),
        }
        for i in range(N_CORES)
    ]
    res = run_bass_kernel_spmd(nc, in_maps, core_ids=list(range(N_CORES)))

    per_img_d = []   # [B] sum(o - d) per image
    sq_total = 0.0
    box_sums = []    # [B, NB]
    for ci, r in enumerate(res.results):
        a = r["acc"].astype(np.float64)
        at = r["acct"].astype(np.float64)
        stt_cols = a[:, :NSTT].sum(axis=0)
        sq_total += a[:, ACC_SQ:ACC_BOX].sum() + at[:, 1].sum() + at[:, 3].sum()
        per_img_d.extend(
            [
                stt_cols[0] + stt_cols[1],
                stt_cols[2] + stt_cols[3],
                stt_cols[4] + stt_cols[5],
                stt_cols[6] + stt_cols[7] + stt_cols[8]
                + at[:, 0].sum() + at[:, 2].sum(),
            ]
        )
        # box partials: [128, (img, cx, j)]; full x index = cx * 128 + p
        bp = a[:, ACC_BOX:].reshape(P, NIMG, NCH, NB)
        for k in range(NIMG):
            img = ci * NIMG + k
            bx = bp[:, k].transpose(1, 0, 2).reshape(W, NB)
            box_sums.append((bx * cm[img]).sum(axis=0))

    dmap_loss = sq_total / float(num_objects)
    count_loss = float(np.mean(np.asarray(per_img_d) ** 2))
    min_count = float(np.maximum(0.0, 1.0 - np.asarray(box_sums)).sum())
    return np.array([dmap_loss, count_loss, min_count], dtype=np.float32)
